# revision 3
# baseline (speedup 1.0000x reference)
"""MLA prefill kernel for Trainium2, 8 NeuronCores.

Sharding: data-parallel over batch (2) x tensor-parallel over heads
(16 heads -> 4 per core).  Core c handles batch c//4, head group c%4.
Each core computes its full attention block plus a partial output
projection; the host sums the 4 per-group partials per batch.

Layout strategy: everything is computed transposed ([feature, L]) so
matmul lhsT/rhs operands are produced directly, except V (L-major for
the PV matmul), which is spilled to a DRAM scratch and re-streamed.
Scores are computed transposed (S^T = K Q^T, [Lk, Lq]) so softmax's
sum runs through the PV matmul via an appended ones-column; exp needs
no max-subtraction (scores are O(10), fp32 exp is safe).  RoPE pair
mixing runs along partitions, done with a +-1 pair-swap matmul (J) on
the tensor engine plus two elementwise multiplies and an add.

Matmuls use float32r (1 cycle/row at N>=512, ~TF32 accuracy).
"""

import math
import os
import sys

sys.path.insert(0, "/opt/trn_rl_repo")

import numpy as np

import concourse.bass as bass
import concourse.mybir as mybir
import concourse.tile as tile
from concourse.bass import ds
from concourse.bass_utils import run_bass_kernel_spmd

H, DH, RK, RD = 16, 128, 512, 64
B, L, E = 2, 2048, 2048
HPG = 4                      # heads per core
NCORE = 8
DV = DH + RD                 # 192
SCALE = 1.0 / math.sqrt(DV)
CH = 512                     # Lq chunk
NCH = L // CH                # 4
LT = L // 128                # 16 key tiles
ET = E // 128                # 16
W1C = HPG * DH + RK + HPG * RD + RD   # 1344 fused QKV columns
VROW = HPG * (DV + 1)        # 772: per-head 192 v dims + ones col

F32 = mybir.dt.float32
F32R = mybir.dt.float32r
AF = mybir.ActivationFunctionType

_CACHE = {}


def _split_excess_waits(nc, limit=1):
    """walrus on this toolchain accepts at most one sem-wait per
    instruction; hoist extras onto same-engine no-ops just before."""
    f = nc.m.functions[0]
    for bb in f.blocks:
        new_list = []
        changed = False
        for inst in bb.instructions:
            si = inst.sync_info
            if si is not None and si.on_wait is not None and len(si.on_wait) > limit:
                waits = list(si.on_wait)
                changed = True
                n = 0
                while len(waits) > limit:
                    chunk, waits = waits[:limit], waits[limit:]
                    new_list.append(mybir.InstNoOp(
                        name=f"{inst.name}-ws{n}",
                        sync_info=mybir.SyncInfo(on_wait=chunk, on_update=[]),
                        bass_nofuse=True,
                        engine=inst.engine,
                    ))
                    n += 1
                inst.sync_info = mybir.SyncInfo(on_wait=waits, on_update=si.on_update)
            new_list.append(inst)
        if changed:
            bb.instructions[:] = new_list
    return nc


def _build():
    nc = bass.Bass(target_bir_lowering=False, trn_type="TRN2")

    xt = nc.dram_tensor("xt", [NCH, 128, ET, CH], F32R, kind="ExternalInput")
    w1 = nc.dram_tensor("w1", [11, 128, ET, 128], F32R, kind="ExternalInput")
    wuk = nc.dram_tensor("wuk", [RK, HPG * DH], F32R, kind="ExternalInput")
    wuv = nc.dram_tensor("wuv", [RK, HPG * DV], F32R, kind="ExternalInput")
    woa = nc.dram_tensor("woa", [E // CH, 128, HPG, CH], F32R, kind="ExternalInput")
    wob = nc.dram_tensor("wob", [E // CH, RD, HPG, CH], F32R, kind="ExternalInput")
    cost = nc.dram_tensor("cost", [128, L], F32R, kind="ExternalInput")
    sint = nc.dram_tensor("sint", [128, L], F32R, kind="ExternalInput")
    jt = nc.dram_tensor("jt", [128, 128], F32R, kind="ExternalInput")
    triu = nc.dram_tensor("triu", [128, 128], F32R, kind="ExternalInput")
    onesc = nc.dram_tensor("onesc", [128, HPG], F32R, kind="ExternalInput")
    outt = nc.dram_tensor("outt", [L, E], F32, kind="ExternalOutput")

    from contextlib import ExitStack

    with tile.TileContext(nc) as tc:
        with ExitStack() as ctx:
            pool_specs = [
                ("consts", 1, None), ("res", 1, None), ("dscr", 1, "DRAM"),
                ("rrd_p", 2, "DRAM"), ("xt_p", 1, None), ("w1_p", 1, None),
                ("qt_p", 2, None), ("rq_p", 2, None), ("ckv_p", 1, None),
                ("vst_p", 4, None), ("vsr_p", 2, None), ("p_p", 3, None),
                ("tmp_p", 1, None), ("rb_p", 1, None), ("ot_p", 1, None),
                ("wo_p", 1, None), ("fin_p", 2, None), ("ps_p", 8, "PSUM"),
            ]
            pools = {}
            for pname, pbufs, pspace in pool_specs:
                kw = {"name": pname, "bufs": pbufs}
                if pspace:
                    kw["space"] = pspace
                pools[pname] = ctx.enter_context(tc.tile_pool(**kw))
            (consts, res, dscr, rrd_p, xt_p, w1_p, qt_p, rq_p, ckv_p, vst_p,
             vsr_p, p_p, tmp_p, rb_p, ot_p, wo_p, fin_p, ps_p) = (
                pools[s[0]] for s in pool_specs)
            def psum():
                return ps_p.tile([128, 512], F32, tag="ps", name="ps")

            # ---- constants / resident weights
            jt_t = consts.tile([128, 128], F32R, tag="jt", name="jt")
            nc.sync.dma_start(out=jt_t[:], in_=jt.ap())
            tri_t = consts.tile([128, 128], F32R, tag="tri", name="tri")
            nc.sync.dma_start(out=tri_t[:], in_=triu.ap())
            wukt = res.tile([128, RK // 128, HPG * DH], F32R, tag="wukt", name="wukt")
            nc.sync.dma_start(out=wukt[:], in_=wuk.ap().rearrange("(t p) n -> p t n", p=128))
            wuvt = res.tile([128, RK // 128, HPG * DV], F32R, tag="wuvt", name="wuvt")
            nc.sync.dma_start(out=wuvt[:], in_=wuv.ap().rearrange("(t p) n -> p t n", p=128))

            ktc = res.tile([128, HPG, L], F32R, tag="ktc", name="ktc")     # K content, transposed
            rkd = res.tile([128, L], F32R, tag="rkd", name="rkd")          # roped k_rope, duplicated rows
            vd = dscr.tile([LT, 128, VROW], F32R, tag="vd", name="vd")    # V spill (L-major + ones)

            # d-tiles of the fused QKV projection: (offset, width, kind, idx)
            dtiles = (
                [(128 * i, 128, "q", i) for i in range(HPG)]
                + [(HPG * DH + 128 * i, 128, "ckv", i) for i in range(RK // 128)]
                + [(HPG * DH + RK + 128 * i, 128, "rq", i) for i in range(2)]
                + [(HPG * DH + RK + HPG * RD, RD, "rk", 0)]
            )

            for c in range(NCH):
                ccols = ds(c * CH, CH)

                # ================= QKV(c): [1344, CH] = W1^T @ x^T =======
                xtt = xt_p.tile([128, ET, CH], F32R, tag="xtt", name="xtt")
                nc.sync.dma_start(out=xtt[:], in_=xt.ap()[c])
                cos_t = rb_p.tile([128, CH], F32R, tag="cosc", name="cosc")
                nc.sync.dma_start(out=cos_t[:], in_=cost.ap()[:, ccols])
                sin_t = rb_p.tile([128, CH], F32R, tag="sinc", name="sinc")
                nc.sync.dma_start(out=sin_t[:], in_=sint.ap()[:, ccols])
                qtc = qt_p.tile([128, HPG, CH], F32R, tag="qtc", name="qtc")
                rq = rq_p.tile([128, 2, CH], F32R, tag="rq", name="rq")
                ckv = ckv_p.tile([128, RK // 128, CH], F32R, tag="ckv", name="ckv")

                for di, (doff, dw, kind, idx) in enumerate(dtiles):
                    w1s = w1_p.tile([128, ET, 128], F32R, tag="w1s", name="w1s")
                    nc.sync.dma_start(out=w1s[:, :, :dw], in_=w1.ap()[di, :, :, :dw])
                    ps = psum()
                    for e in range(ET):
                        nc.tensor.matmul(ps[:dw, :CH], w1s[:, e, :dw], xtt[:, e, :],
                                         start=(e == 0), stop=(e == ET - 1))
                    if kind == "q":
                        nc.scalar.copy(out=qtc[:, idx, :], in_=ps[:, :CH])
                    elif kind == "ckv":
                        nc.vector.tensor_copy(ckv[:, idx, :], ps[:, :CH])
                    elif kind == "rq":
                        nc.vector.tensor_copy(rq[:, idx, :], ps[:, :CH])
                    else:  # pre-rope k_rope at partitions 0:64
                        nc.vector.tensor_copy(rkd[0:RD, ccols], ps[:RD, :CH])

                # ================= RoPE(c) ===============================
                # roped = R * cos + (J @ R) * sin   (pairs along partitions)
                for i in range(2):  # q_rope, two head-pair tiles
                    swp = psum()
                    nc.tensor.matmul(swp[:, :CH], jt_t[:, :], rq[:, i, :],
                                     start=True, stop=True)
                    t1 = tmp_p.tile([128, CH], F32R, tag="ropet", name="ropet")
                    nc.vector.tensor_mul(t1[:], rq[:, i, :], cos_t[:])
                    nc.vector.tensor_mul(rq[:, i, :], swp[:, :CH], sin_t[:])
                    nc.vector.tensor_add(rq[:, i, :], rq[:, i, :], t1[:])
                swp = psum()
                nc.tensor.matmul(swp[:RD, :CH], jt_t[:RD, :RD], rkd[0:RD, ccols],
                                 start=True, stop=True)
                t1 = tmp_p.tile([128, CH], F32R, tag="ropet", name="ropet")
                nc.vector.tensor_mul(t1[:RD, :], rkd[0:RD, ccols], cos_t[0:RD, :])
                nc.vector.tensor_mul(rkd[0:RD, ccols], swp[:RD, :CH], sin_t[0:RD, :])
                nc.vector.tensor_add(rkd[0:RD, ccols], rkd[0:RD, ccols], t1[:RD, :])
                # duplicate roped k_rope to partitions 64:128 (for odd heads)
                nc.sync.dma_start(out=rkd[RD:128, ccols], in_=rkd[0:RD, ccols])

                # ================= UP-K(c): K^T = Wuk^T @ c_kv^T =========
                for h in range(HPG):
                    ps = psum()
                    for kt in range(RK // 128):
                        nc.tensor.matmul(ps[:, :CH], wukt[:, kt, ds(128 * h, 128)],
                                         ckv[:, kt, :],
                                         start=(kt == 0), stop=(kt == RK // 128 - 1))
                    nc.scalar.copy(out=ktc[:, h, ccols], in_=ps[:, :CH])

                # ================= UP-V(c): V = c_kv @ Wuv (L-major) =====
                chunk_vst = []
                for lti in range(4):
                    lt = 4 * c + lti
                    vst = vst_p.tile([128, VROW], F32R, tag="vst", name="vst")
                    chunk_vst.append(vst)
                    for nb in range(2):
                        psv = psum()
                        for kt in range(RK // 128):
                            nc.tensor.matmul(psv[:, :384],
                                             ckv[:, kt, ds(128 * lti, 128)],
                                             wuvt[:, kt, ds(384 * nb, 384)],
                                             start=(kt == 0), stop=(kt == RK // 128 - 1))
                        for q in range(2):
                            hh = 2 * nb + q
                            nc.scalar.copy(out=vst[:, ds((DV + 1) * hh, DV)],
                                           in_=psv[:, ds(DV * q, DV)])
                    ones_view = vst[:].rearrange("p (h x) -> p h x", x=DV + 1)
                    nc.sync.dma_start(out=ones_view[:, :, DV], in_=onesc.ap())
                    nc.sync.dma_start(out=vd[lt], in_=vst[:])

                # ================= ATT(c): head pairs ====================
                ntk = 4 * c + 4
                for hp in range(2):
                    pvs = []
                    for q in range(2):
                        pvs.append((psum(), psum()))  # (ps1, ps2) per head
                    for t in range(ntk):
                        j = t - 4 * c
                        off = 128 * j if j >= 0 else 0
                        n = CH - off
                        if j >= 0:
                            vsr = chunk_vst[j]
                        else:
                            vsr = vsr_p.tile([128, VROW], F32R, tag="vsr", name="vsr")
                            nc.gpsimd.dma_start(out=vsr[:], in_=vd[t])
                        for q in range(2):
                            h = 2 * hp + q
                            ps1, ps2 = pvs[q]
                            hb = RD * (h % 2)
                            sps = psum()
                            nc.tensor.matmul(sps[:, ds(off, n)],
                                             ktc[:, h, ds(128 * t, 128)],
                                             qtc[:, h, ds(off, n)],
                                             start=True, stop=False)
                            nc.tensor.matmul(sps[:, ds(off, n)],
                                             rkd[hb:hb + RD, ds(128 * t, 128)],
                                             rq[hb:hb + RD, h // 2, ds(off, n)],
                                             start=False, stop=True)
                            pt = p_p.tile([128, CH], F32R, tag="pt", name="pt")
                            nc.scalar.activation(pt[:, ds(off, n)], sps[:, ds(off, n)],
                                                 AF.Exp, scale=SCALE)
                            if j >= 0:
                                nc.vector.tensor_mul(pt[:, ds(off, 128)],
                                                     pt[:, ds(off, 128)], tri_t[:])
                            nc.tensor.matmul(ps1[:, ds(off, n)],
                                             vsr[:, ds((DV + 1) * h, 128)],
                                             pt[:, ds(off, n)],
                                             start=(t == 0), stop=(t == ntk - 1),
                                             skip_group_check=True)
                            nc.tensor.matmul(ps2[:DV - DH + 1, ds(off, n)],
                                             vsr[:, ds((DV + 1) * h + DH, DV - DH + 1)],
                                             pt[:, ds(off, n)],
                                             start=(t == 0), stop=(t == ntk - 1),
                                             skip_group_check=True)
                    if hp == 0:
                        ota = ot_p.tile([128, HPG, CH], F32R, tag="ota", name="ota")
                        otb = ot_p.tile([RD, HPG, CH], F32R, tag="otb", name="otb")
                    for q in range(2):
                        h = 2 * hp + q
                        ps1, ps2 = pvs[q]
                        rr = tmp_p.tile([128, CH], F32R, tag="rr", name="rr")
                        with nc.allow_low_precision(reason="softmax denom recip in f32r"):
                            nc.vector.reciprocal(rr[RD:RD + 1, :], ps2[RD:RD + 1, :CH])
                        rrd = rrd_p.tile([1, CH], F32R, tag="rrd", name="rrd")
                        nc.sync.dma_start(out=rrd[:], in_=rr[RD:RD + 1, :])
                        rb = rb_p.tile([128, CH], F32R, tag="rb", name="rb")
                        nc.sync.dma_start(
                            out=rb[:],
                            in_=bass.AP(tensor=rrd.tensor, offset=rrd.offset,
                                        ap=[[0, 128]] + list(rrd.ap[1:])))
                        nc.vector.tensor_mul(ota[:, h, :], ps1[:, :CH], rb[:])
                        nc.vector.tensor_mul(otb[:, h, :], ps2[0:RD, :CH], rb[0:RD, :])

                # ===== FINAL(c): out = attn @ WO, L-major (W-moving) =====
                for eg in range(E // CH):
                    woall = wo_p.tile([128, HPG, CH], F32R, tag="woall", name="woall")
                    nc.scalar.dma_start(out=woall[:], in_=woa.ap()[eg])
                    woallb = wo_p.tile([RD, HPG, CH], F32R, tag="woallb", name="woallb")
                    nc.scalar.dma_start(out=woallb[:], in_=wob.ap()[eg])
                    wts = [(woall[:, kt, :], woallb[:, kt, :]) for kt in range(HPG)]
                    for ls in range(CH // 128):
                        fps = psum()
                        for kt in range(HPG):
                            nc.tensor.matmul(fps[:, :CH], ota[:, kt, ds(128 * ls, 128)],
                                             wts[kt][0],
                                             start=(kt == 0), stop=False,
                                             skip_group_check=True)
                        for kt in range(HPG):
                            nc.tensor.matmul(fps[:, :CH], otb[:, kt, ds(128 * ls, 128)],
                                             wts[kt][1],
                                             start=False, stop=(kt == HPG - 1),
                                             skip_group_check=True)
                        fin = fin_p.tile([128, CH], F32, tag="fin", name="fin")
                        nc.scalar.copy(out=fin[:], in_=fps[:, :CH])
                        nc.gpsimd.dma_start(
                            out=outt.ap()[ds(c * CH + 128 * ls, 128), ds(CH * eg, CH)],
                            in_=fin[:])

    _split_excess_waits(nc)
    return nc


def _prep_inputs(x, cos_table, sin_table, wq, wkv_down, w_up, w_out):
    f32 = np.float32
    wq3 = np.asarray(wq, f32).reshape(E, H, DV)
    wup3 = np.asarray(w_up, f32).reshape(RK, H, 2 * DH + RD)
    wo3 = np.asarray(w_out, f32).reshape(H, DV, E)
    wkv = np.asarray(wkv_down, f32)

    cosI = np.repeat(np.asarray(cos_table, f32)[:L], 2, axis=1).T  # [64, L]
    sinI = np.repeat(np.asarray(sin_table, f32)[:L], 2, axis=1).T
    cost = np.ascontiguousarray(np.concatenate([cosI, cosI], 0))   # [128, L]
    sint = np.ascontiguousarray(np.concatenate([sinI, sinI], 0))
    J = np.zeros((128, 128), f32)
    for i in range(64):
        J[2 * i, 2 * i + 1] = -1.0
        J[2 * i + 1, 2 * i] = 1.0
    jt = np.ascontiguousarray(J.T)
    triu = np.ascontiguousarray(np.triu(np.ones((128, 128), f32)))

    in_maps = []
    for core in range(NCORE):
        b, g = core // HPG, core % HPG
        hs = slice(HPG * g, HPG * g + HPG)
        xT = np.asarray(x, f32)[b].T                       # [E, L]
        xt_pack = np.ascontiguousarray(
            xT.reshape(ET, 128, NCH, CH).transpose(2, 1, 0, 3))  # [NCH,128,ET,CH]
        wq_c = wq3[:, hs, :DH].reshape(E, HPG * DH)
        wq_r = wq3[:, hs, DH:].reshape(E, HPG * RD)
        w1_flat = np.concatenate([wq_c, wkv[:, :RK], wq_r, wkv[:, RK:]], axis=1)
        # pack into 11 d-strips [128, ET, 128] (last strip: 64 cols, zero-pad)
        woa_flat = wo3[hs, :DH, :].reshape(HPG, DH, E)     # [4,128,E]
        woa_pack = np.ascontiguousarray(
            woa_flat.reshape(HPG, DH, E // 512, 512).transpose(2, 1, 0, 3))
        wob_flat = wo3[hs, DH:, :].reshape(HPG, RD, E)
        wob_pack = np.ascontiguousarray(
            wob_flat.reshape(HPG, RD, E // 512, 512).transpose(2, 1, 0, 3))
        w1_pack = np.zeros((11, 128, ET, 128), f32)
        offs = [128 * i for i in range(10)] + [1280]
        wids = [128] * 10 + [64]
        for di, (o, w) in enumerate(zip(offs, wids)):
            w1_pack[di, :, :, :w] = (
                w1_flat[:, o:o + w].reshape(ET, 128, w).transpose(1, 0, 2))
        in_maps.append({
            "xt": xt_pack,
            "w1": np.ascontiguousarray(w1_pack),
            "wuk": np.ascontiguousarray(wup3[:, hs, :DH].reshape(RK, HPG * DH)),
            "wuv": np.ascontiguousarray(wup3[:, hs, DH:].reshape(RK, HPG * DV)),
            "woa": woa_pack,
            "wob": wob_pack,
            "cost": cost,
            "sint": sint,
            "jt": jt,
            "onesc": np.ones((128, HPG), f32),
            "triu": triu,
        })
    return in_maps


def kernel(x, cos_table, sin_table, wq, wkv_down, w_up, w_out, _want_perf=False):
    if "nc" not in _CACHE:
        _CACHE["nc"] = _build()
    nc = _CACHE["nc"]
    in_maps = _prep_inputs(x, cos_table, sin_table, wq, wkv_down, w_up, w_out)
    res = run_bass_kernel_spmd(nc, in_maps, core_ids=list(range(NCORE)),
                               trace=bool(_want_perf),
                               tmpdir=os.environ.get("BASS_TMPDIR") or None)
    out = np.zeros((B, L, E), np.float32)
    for core in range(NCORE):
        b = core // HPG
        out[b] += res.results[core]["outt"]
    if _want_perf:
        return out, res
    return out



# revision 9
# speedup vs baseline: 2.1571x; 2.1571x over previous
"""MLA prefill kernel for Trainium2, 8 NeuronCores.

Sharding: data-parallel over batch (2) x tensor-parallel over heads
(16 heads -> 4 per core).  Core c handles batch c//4, head group c%4.
Each core computes its full attention block plus a partial output
projection; the host sums the 4 per-group partials per batch.

All matmul operands are bf16 (1 cycle/row on the PE; fp32/f32r ran in
fp32_mode=HIGH at ~2.5 cycles/row), accumulation stays f32 in PSUM.
Everything is computed transposed ([feature, L]) so matmul lhsT/rhs
operands are produced directly, except V (L-major for the PV matmul),
which stays resident in SBUF.  Scores are computed transposed
(S^T = K Q^T, [Lk, Lq]) so softmax's sum runs through the PV matmul
via an appended ones-column; exp needs no max-subtraction (scores are
O(10)).  RoPE pair mixing runs along partitions via a +-1 pair-swap
matmul (J) plus two elementwise multiplies and an add.

The attention inner loop is software-pipelined one key-tile ahead
(QK(t+1) issues before PV(t)) so the tensor engine does not stall on
the exp activation.  PSUM: 4 banks hold the two heads' PV accumulators,
4 banks rotate for scores/projections.  The softmax denominator row is
reciprocal'd on the DVE and broadcast across partitions with a K=1
ones matmul.  w_out is SBUF-resident with the per-head rope halves
packed in pairs so the output projection runs 6 full-K=128 matmuls.
"""

import math
import os
import sys

sys.path.insert(0, "/opt/trn_rl_repo")

import numpy as np
import ml_dtypes

import concourse.bass as bass
import concourse.mybir as mybir
import concourse.tile as tile
from concourse.bass import ds
from concourse.bass_utils import run_bass_kernel_spmd

H, DH, RK, RD = 16, 128, 512, 64
B, L, E = 2, 2048, 2048
HPG = 4                      # heads per core
NCORE = 8
DV = DH + RD                 # 192
SCALE = 1.0 / math.sqrt(DV)
CH = 512                     # Lq chunk
NCH = L // CH                # 4
LT = L // 128                # 16 key tiles
ET = E // 128                # 16
VROW = HPG * (DV + 1)        # 772: per-head 192 v dims + ones col

F32 = mybir.dt.float32
BF16 = mybir.dt.bfloat16
AF = mybir.ActivationFunctionType
NPBF = ml_dtypes.bfloat16

_CACHE = {}


def _split_excess_waits(nc, limit=1):
    """walrus on this toolchain accepts at most one sem-wait per
    instruction; hoist extras onto same-engine no-ops just before."""
    f = nc.m.functions[0]
    for bb in f.blocks:
        new_list = []
        changed = False
        for inst in bb.instructions:
            si = inst.sync_info
            if si is not None and si.on_wait is not None and len(si.on_wait) > limit:
                waits = list(si.on_wait)
                changed = True
                n = 0
                while len(waits) > limit:
                    chunk, waits = waits[:limit], waits[limit:]
                    new_list.append(mybir.InstNoOp(
                        name=f"{inst.name}-ws{n}",
                        sync_info=mybir.SyncInfo(on_wait=chunk, on_update=[]),
                        bass_nofuse=True,
                        engine=inst.engine,
                    ))
                    n += 1
                inst.sync_info = mybir.SyncInfo(on_wait=waits, on_update=si.on_update)
            new_list.append(inst)
        if changed:
            bb.instructions[:] = new_list
    return nc


def _build():
    nc = bass.Bass(target_bir_lowering=False, trn_type="TRN2")

    xt = nc.dram_tensor("xt", [NCH, 128, ET, CH], BF16, kind="ExternalInput")
    w1 = nc.dram_tensor("w1", [11, 128, ET, 128], BF16, kind="ExternalInput")
    wuk = nc.dram_tensor("wuk", [RK, HPG * DH], BF16, kind="ExternalInput")
    wuv = nc.dram_tensor("wuv", [RK, HPG * DV], BF16, kind="ExternalInput")
    wo = nc.dram_tensor("wo", [128, 6, E], BF16, kind="ExternalInput")
    cost = nc.dram_tensor("cost", [128, L], BF16, kind="ExternalInput")
    sint = nc.dram_tensor("sint", [128, L], BF16, kind="ExternalInput")
    jt = nc.dram_tensor("jt", [128, 128], BF16, kind="ExternalInput")
    triu = nc.dram_tensor("triu", [128, 128], BF16, kind="ExternalInput")
    ones1 = nc.dram_tensor("ones1", [1, 128], BF16, kind="ExternalInput")
    outt = nc.dram_tensor("outt", [L, E], BF16, kind="ExternalOutput")

    from contextlib import ExitStack

    with tile.TileContext(nc) as tc:
        with ExitStack() as ctx:
            ctx.enter_context(nc.allow_low_precision(
                reason="bf16 kernel; all contractions accumulate in f32 psum"))
            pool_specs = [
                ("consts", 1, None), ("res", 1, None),
                ("xt_p", 2, None), ("w1_p", 4, None),
                ("qt_p", 2, None), ("rq_p", 2, None), ("ckv_p", 2, None),
                ("pt_p", 6, None), ("tmp_p", 2, None),
                ("oz_p", 2, None), ("fin_p", 3, None),
                ("acc_p", 2, "PSUM"), ("rot_p", 4, "PSUM"),
            ]
            pools = {}
            for pname, pbufs, pspace in pool_specs:
                kw = {"name": pname, "bufs": pbufs}
                if pspace:
                    kw["space"] = pspace
                pools[pname] = ctx.enter_context(tc.tile_pool(**kw))
            (consts, res, xt_p, w1_p, qt_p, rq_p, ckv_p, pt_p, tmp_p,
             oz_p, fin_p, acc_p, rot_p) = (pools[s[0]] for s in pool_specs)

            def rot():
                return rot_p.tile([128, 512], F32, tag="ps", name="ps")

            # ---- constants / resident weights
            jt_t = consts.tile([128, 128], BF16, tag="jt", name="jt")
            nc.sync.dma_start(out=jt_t[:], in_=jt.ap())
            tri_t = consts.tile([128, 128], BF16, tag="tri", name="tri")
            nc.sync.dma_start(out=tri_t[:], in_=triu.ap())
            one_t = consts.tile([1, 128], BF16, tag="one", name="one")
            nc.sync.dma_start(out=one_t[:], in_=ones1.ap())
            wukt = res.tile([128, RK // 128, HPG * DH], BF16, tag="wukt", name="wukt")
            nc.sync.dma_start(out=wukt[:], in_=wuk.ap().rearrange("(t p) n -> p t n", p=128))
            wuvt = res.tile([128, RK // 128, HPG * DV], BF16, tag="wuvt", name="wuvt")
            nc.sync.dma_start(out=wuvt[:], in_=wuv.ap().rearrange("(t p) n -> p t n", p=128))
            wo_t = res.tile([128, 6, E], BF16, tag="wo", name="wo")
            nc.scalar.dma_start(out=wo_t[:], in_=wo.ap())
            cos_sb = res.tile([128, L], BF16, tag="cos", name="cos")
            nc.scalar.dma_start(out=cos_sb[:], in_=cost.ap())
            sin_sb = res.tile([128, L], BF16, tag="sin", name="sin")
            nc.scalar.dma_start(out=sin_sb[:], in_=sint.ap())

            ktc = res.tile([128, HPG, L], BF16, tag="ktc", name="ktc")   # K content, transposed
            rkd = res.tile([128, L], BF16, tag="rkd", name="rkd")        # roped k_rope, dup rows
            vd = res.tile([128, LT, VROW], BF16, tag="vd", name="vd")    # V resident (L-major + ones)
            vdv = vd[:].rearrange("p t (h x) -> p t h x", x=DV + 1)
            nc.gpsimd.memset(vdv[:, :, :, DV], 1.0)                      # ones columns

            # d-tiles of the fused QKV projection: (kind, idx)
            dtiles = ([("q", i) for i in range(HPG)]
                      + [("ckv", i) for i in range(RK // 128)]
                      + [("rq", i) for i in range(2)]
                      + [("rk", 0)])

            for c in range(NCH):
                ccols = ds(c * CH, CH)

                # ================= QKV(c): [1344, CH] = W1^T @ x^T =======
                xtt = xt_p.tile([128, ET, CH], BF16, tag="xtt", name="xtt")
                nc.sync.dma_start(out=xtt[:], in_=xt.ap()[c])
                qtc = qt_p.tile([128, HPG, CH], BF16, tag="qtc", name="qtc")
                rq = rq_p.tile([128, 2, CH], BF16, tag="rq", name="rq")
                ckv = ckv_p.tile([128, RK // 128, CH], BF16, tag="ckv", name="ckv")

                for di, (kind, idx) in enumerate(dtiles):
                    w1s = w1_p.tile([128, ET, 128], BF16, tag="w1s", name="w1s")
                    nc.sync.dma_start(out=w1s[:], in_=w1.ap()[di])
                    dw = RD if kind == "rk" else 128
                    ps = rot()
                    for e in range(ET):
                        nc.tensor.matmul(ps[:dw, :CH], w1s[:, e, :dw], xtt[:, e, :],
                                         start=(e == 0), stop=(e == ET - 1))
                    if kind == "q":
                        nc.scalar.copy(out=qtc[:, idx, :], in_=ps[:, :CH])
                    elif kind == "ckv":
                        nc.vector.tensor_copy(ckv[:, idx, :], ps[:, :CH])
                    elif kind == "rq":
                        nc.vector.tensor_copy(rq[:, idx, :], ps[:, :CH])
                    else:  # pre-rope k_rope at partitions 0:64
                        nc.vector.tensor_copy(rkd[0:RD, ccols], ps[:RD, :CH])

                # ================= RoPE(c) ===============================
                # roped = R * cos + (J @ R) * sin   (pairs along partitions)
                for i in range(2):  # q_rope, two head-pair tiles
                    swp = rot()
                    nc.tensor.matmul(swp[:, :CH], jt_t[:, :], rq[:, i, :],
                                     start=True, stop=True)
                    t1 = tmp_p.tile([128, CH], BF16, tag="ropet", name="ropet")
                    nc.vector.tensor_mul(t1[:], rq[:, i, :], cos_sb[:, ccols])
                    nc.vector.tensor_mul(rq[:, i, :], swp[:, :CH], sin_sb[:, ccols])
                    nc.vector.tensor_add(rq[:, i, :], rq[:, i, :], t1[:])
                swp = rot()
                nc.tensor.matmul(swp[:RD, :CH], jt_t[:RD, :RD], rkd[0:RD, ccols],
                                 start=True, stop=True)
                t1 = tmp_p.tile([128, CH], BF16, tag="ropet", name="ropet")
                nc.vector.tensor_mul(t1[:RD, :], rkd[0:RD, ccols], cos_sb[0:RD, ccols])
                nc.vector.tensor_mul(rkd[0:RD, ccols], swp[:RD, :CH], sin_sb[0:RD, ccols])
                nc.vector.tensor_add(rkd[0:RD, ccols], rkd[0:RD, ccols], t1[:RD, :])
                # duplicate roped k_rope to partitions 64:128 (for odd heads)
                nc.gpsimd.dma_start(out=rkd[RD:128, ccols], in_=rkd[0:RD, ccols])

                # ================= UP-K(c): K^T = Wuk^T @ c_kv^T =========
                for h in range(HPG):
                    ps = rot()
                    for kt in range(RK // 128):
                        nc.tensor.matmul(ps[:, :CH], wukt[:, kt, ds(128 * h, 128)],
                                         ckv[:, kt, :],
                                         start=(kt == 0), stop=(kt == RK // 128 - 1))
                    nc.scalar.copy(out=ktc[:, h, ccols], in_=ps[:, :CH])

                # ================= UP-V(c): V = c_kv @ Wuv (L-major) =====
                for lti in range(4):
                    lt = 4 * c + lti
                    for nb in range(2):
                        psv = rot()
                        for kt in range(RK // 128):
                            nc.tensor.matmul(psv[:, :384],
                                             ckv[:, kt, ds(128 * lti, 128)],
                                             wuvt[:, kt, ds(384 * nb, 384)],
                                             start=(kt == 0), stop=(kt == RK // 128 - 1))
                        for q in range(2):
                            hh = 2 * nb + q
                            nc.vector.tensor_copy(vd[:, lt, ds((DV + 1) * hh, DV)],
                                                  psv[:, ds(DV * q, DV)])

                # ================= ATT(c): head pairs, 1-tile pipelined ==
                ntk = 4 * c + 4
                oz = oz_p.tile([128, 6, CH], BF16, tag="oz", name="oz")
                for hp in range(2):
                    heads = (2 * hp, 2 * hp + 1)
                    A = [(acc_p.tile([128, 512], F32, tag="acc1", name="acc1"),
                          acc_p.tile([128, 512], F32, tag="acc2", name="acc2"))
                         for _ in range(2)]
                    geom = []
                    for t in range(ntk):
                        j = t - 4 * c
                        off = 128 * j if j >= 0 else 0
                        geom.append((off, CH - off, j >= 0))
                    sps_l = {}
                    pt_l = {}

                    def emit_qk(t):
                        off, n, _ = geom[t]
                        sps_l[t] = []
                        for q in range(2):
                            h = heads[q]
                            hb = RD * (h % 2)
                            sps = rot()
                            nc.tensor.matmul(sps[:, ds(off, n)],
                                             ktc[:, h, ds(128 * t, 128)],
                                             qtc[:, h, ds(off, n)],
                                             start=True, stop=False)
                            nc.tensor.matmul(sps[:, ds(off, n)],
                                             rkd[hb:hb + RD, ds(128 * t, 128)],
                                             rq[hb:hb + RD, h // 2, ds(off, n)],
                                             start=False, stop=True)
                            sps_l[t].append(sps)

                    def emit_exp(t):
                        off, n, diag = geom[t]
                        pt_l[t] = []
                        for q in range(2):
                            pt = pt_p.tile([128, CH], BF16, tag="pt", name="pt")
                            nc.scalar.activation(pt[:, ds(off, n)],
                                                 sps_l[t][q][:, ds(off, n)],
                                                 AF.Exp, scale=SCALE)
                            if diag:
                                nc.vector.tensor_mul(pt[:, ds(off, 128)],
                                                     pt[:, ds(off, 128)], tri_t[:])
                            pt_l[t].append(pt)

                    def emit_pv(t):
                        off, n, _ = geom[t]
                        for q in range(2):
                            h = heads[q]
                            ps1, ps2 = A[q]
                            pt = pt_l[t][q]
                            nc.tensor.matmul(ps1[:, ds(off, n)],
                                             vd[:, t, ds((DV + 1) * h, 128)],
                                             pt[:, ds(off, n)],
                                             start=(t == 0), stop=(t == ntk - 1),
                                             skip_group_check=True)
                            nc.tensor.matmul(ps2[:DV - DH + 1, ds(off, n)],
                                             vd[:, t, ds((DV + 1) * h + DH, DV - DH + 1)],
                                             pt[:, ds(off, n)],
                                             start=(t == 0), stop=(t == ntk - 1),
                                             skip_group_check=True)

                    # 1-tile software pipeline: QK(t+1) issues before PV(t)
                    emit_qk(0)
                    emit_exp(0)
                    for t in range(1, ntk):
                        emit_qk(t)
                        emit_pv(t - 1)
                        emit_exp(t)
                    emit_pv(ntk - 1)

                    # ---- softmax denominators + normalize into oz
                    for q in range(2):
                        h = heads[q]
                        ps1, ps2 = A[q]
                        rr16 = tmp_p.tile([1, CH], BF16, tag="rr16", name="rr16")
                        nc.vector.reciprocal(rr16[:], ps2[RD:RD + 1, :CH])
                        rb = rot()
                        nc.tensor.matmul(rb[:, :CH], one_t[:, :], rr16[:, :],
                                         start=True, stop=True)
                        rbs = tmp_p.tile([128, CH], BF16, tag="rbs", name="rbs")
                        nc.scalar.copy(out=rbs[:], in_=rb[:, :CH])
                        nc.vector.tensor_mul(oz[:, h, :], ps1[:, :CH], rbs[:])
                        hb = RD * (h % 2)
                        nc.vector.tensor_mul(oz[hb:hb + RD, 4 + hp, :],
                                             ps2[0:RD, :CH], rbs[0:RD, :])

                # ===== FINAL(c): out = attn @ WO, L-major (W-moving) =====
                for eg in range(E // CH):
                    for ls in range(CH // 128):
                        fps = rot()
                        for kt in range(6):
                            nc.tensor.matmul(fps[:, :CH],
                                             oz[:, kt, ds(128 * ls, 128)],
                                             wo_t[:, kt, ds(CH * eg, CH)],
                                             start=(kt == 0), stop=(kt == 5))
                        fin = fin_p.tile([128, CH], BF16, tag="fin", name="fin")
                        nc.scalar.copy(out=fin[:], in_=fps[:, :CH])
                        nc.gpsimd.dma_start(
                            out=outt.ap()[ds(c * CH + 128 * ls, 128), ds(CH * eg, CH)],
                            in_=fin[:])

    _split_excess_waits(nc)
    return nc


def _prep_inputs(x, cos_table, sin_table, wq, wkv_down, w_up, w_out):
    f32 = np.float32
    wq3 = np.asarray(wq, f32).reshape(E, H, DV)
    wup3 = np.asarray(w_up, f32).reshape(RK, H, 2 * DH + RD)
    wo3 = np.asarray(w_out, f32).reshape(H, DV, E)
    wkv = np.asarray(wkv_down, f32)

    cosI = np.repeat(np.asarray(cos_table, f32)[:L], 2, axis=1).T  # [64, L]
    sinI = np.repeat(np.asarray(sin_table, f32)[:L], 2, axis=1).T
    cost = np.ascontiguousarray(np.concatenate([cosI, cosI], 0)).astype(NPBF)
    sint = np.ascontiguousarray(np.concatenate([sinI, sinI], 0)).astype(NPBF)
    J = np.zeros((128, 128), f32)
    for i in range(64):
        J[2 * i, 2 * i + 1] = -1.0
        J[2 * i + 1, 2 * i] = 1.0
    jt = np.ascontiguousarray(J.T).astype(NPBF)
    triu = np.ascontiguousarray(np.triu(np.ones((128, 128), f32))).astype(NPBF)

    in_maps = []
    for core in range(NCORE):
        b, g = core // HPG, core % HPG
        hs = slice(HPG * g, HPG * g + HPG)
        xT = np.asarray(x, f32)[b].T                       # [E, L]
        xt_pack = np.ascontiguousarray(
            xT.reshape(ET, 128, NCH, CH).transpose(2, 1, 0, 3)).astype(NPBF)
        wq_c = wq3[:, hs, :DH].reshape(E, HPG * DH)
        wq_r = wq3[:, hs, DH:].reshape(E, HPG * RD)
        w1_flat = np.concatenate([wq_c, wkv[:, :RK], wq_r, wkv[:, RK:]], axis=1)
        # pack into 11 d-strips [128, ET, 128] (last strip: 64 cols, zero-pad)
        w1_pack = np.zeros((11, 128, ET, 128), f32)
        offs = [128 * i for i in range(10)] + [1280]
        wids = [128] * 10 + [64]
        for di, (o, w) in enumerate(zip(offs, wids)):
            w1_pack[di, :, :, :w] = (
                w1_flat[:, o:o + w].reshape(ET, 128, w).transpose(1, 0, 2))
        # w_out resident: 4 content strips + 2 rope-pair strips
        wo_pack = np.zeros((128, 6, E), f32)
        wog = wo3[hs]                                      # [4, 192, E]
        for kt in range(HPG):
            wo_pack[:, kt, :] = wog[kt, :DH, :]
        for hp in range(2):
            wo_pack[0:RD, 4 + hp, :] = wog[2 * hp, DH:, :]
            wo_pack[RD:128, 4 + hp, :] = wog[2 * hp + 1, DH:, :]
        in_maps.append({
            "xt": xt_pack,
            "w1": w1_pack.astype(NPBF),
            "wuk": np.ascontiguousarray(
                wup3[:, hs, :DH].reshape(RK, HPG * DH)).astype(NPBF),
            "wuv": np.ascontiguousarray(
                wup3[:, hs, DH:].reshape(RK, HPG * DV)).astype(NPBF),
            "wo": wo_pack.astype(NPBF),
            "cost": cost,
            "sint": sint,
            "jt": jt,
            "triu": triu,
            "ones1": np.ones((1, 128), NPBF),
        })
    return in_maps


def kernel(x, cos_table, sin_table, wq, wkv_down, w_up, w_out, _want_perf=False):
    if "nc" not in _CACHE:
        _CACHE["nc"] = _build()
    nc = _CACHE["nc"]
    in_maps = _prep_inputs(x, cos_table, sin_table, wq, wkv_down, w_up, w_out)
    res = run_bass_kernel_spmd(nc, in_maps, core_ids=list(range(NCORE)),
                               trace=bool(_want_perf),
                               tmpdir=os.environ.get("BASS_TMPDIR") or None)
    out = np.zeros((B, L, E), np.float32)
    for core in range(NCORE):
        b = core // HPG
        out[b] += res.results[core]["outt"].astype(np.float32)
    if _want_perf:
        return out, res
    return out


# revision 14
# speedup vs baseline: 2.3345x; 1.0823x over previous
"""MLA prefill kernel (fp16) for Trainium2, 8 NeuronCores.

Sharding: data-parallel over batch (2) x tensor-parallel over heads
(16 heads -> 4 per core).  Core c handles batch c//4, head group c%4.
Each core computes its full attention block plus a partial output
projection; the host sums the 4 per-group partials per batch.

All matmul operands are bf16 (1 cycle/row on the PE; fp32/f32r ran in
fp32_mode=HIGH at ~2.5 cycles/row), accumulation stays f32 in PSUM.
Everything is computed transposed ([feature, L]) so matmul lhsT/rhs
operands are produced directly, except V (L-major for the PV matmul),
which stays resident in SBUF.  Scores are computed transposed
(S^T = K Q^T, [Lk, Lq]) so softmax's sum runs through the PV matmul
via an appended ones-column; exp needs no max-subtraction (scores are
O(10)).  RoPE pair mixing runs along partitions via a +-1 pair-swap
matmul (J) plus two elementwise multiplies and an add.

The attention inner loop is software-pipelined one key-tile ahead
(QK(t+1) issues before PV(t)) so the tensor engine does not stall on
the exp activation.  PSUM: 4 banks hold the two heads' PV accumulators,
4 banks rotate for scores/projections.  The softmax denominator row is
reciprocal'd on the DVE and broadcast across partitions with a K=1
ones matmul.  w_out is SBUF-resident with the per-head rope halves
packed in pairs so the output projection runs 6 full-K=128 matmuls.
"""

import math
import os
import sys

sys.path.insert(0, "/opt/trn_rl_repo")

import numpy as np
import ml_dtypes

import concourse.bass as bass
import concourse.mybir as mybir
import concourse.tile as tile
from concourse.bass import ds
from concourse.bass_utils import run_bass_kernel_spmd

H, DH, RK, RD = 16, 128, 512, 64
B, L, E = 2, 2048, 2048
HPG = 4                      # heads per core
NCORE = 8
DV = DH + RD                 # 192
SCALE = 1.0 / math.sqrt(DV)
CH = 512                     # Lq chunk
NCH = L // CH                # 4
LT = L // 128                # 16 key tiles
ET = E // 128                # 16
VROW = HPG * (DV + 1)        # 772: per-head 192 v dims + ones col

F32 = mybir.dt.float32
FP16 = mybir.dt.float16
AF = mybir.ActivationFunctionType
NPFP16 = np.float16

_CACHE = {}


def _split_excess_waits(nc, limit=1):
    """walrus on this toolchain accepts at most one sem-wait per
    instruction; hoist extras onto same-engine no-ops just before."""
    f = nc.m.functions[0]
    for bb in f.blocks:
        new_list = []
        changed = False
        for inst in bb.instructions:
            si = inst.sync_info
            if si is not None and si.on_wait is not None and len(si.on_wait) > limit:
                waits = list(si.on_wait)
                changed = True
                n = 0
                while len(waits) > limit:
                    chunk, waits = waits[:limit], waits[limit:]
                    new_list.append(mybir.InstNoOp(
                        name=f"{inst.name}-ws{n}",
                        sync_info=mybir.SyncInfo(on_wait=chunk, on_update=[]),
                        bass_nofuse=True,
                        engine=inst.engine,
                    ))
                    n += 1
                inst.sync_info = mybir.SyncInfo(on_wait=waits, on_update=si.on_update)
            new_list.append(inst)
        if changed:
            bb.instructions[:] = new_list
    return nc


def _build():
    nc = bass.Bass(target_bir_lowering=False, trn_type="TRN2")

    xt = nc.dram_tensor("xt", [NCH, 128, ET, CH], FP16, kind="ExternalInput")
    w1 = nc.dram_tensor("w1", [11, 128, ET, 128], FP16, kind="ExternalInput")
    wuk = nc.dram_tensor("wuk", [RK, HPG * DH], FP16, kind="ExternalInput")
    wuv = nc.dram_tensor("wuv", [RK, HPG * DV], FP16, kind="ExternalInput")
    wo = nc.dram_tensor("wo", [128, 6, E], FP16, kind="ExternalInput")
    cost = nc.dram_tensor("cost", [128, L], FP16, kind="ExternalInput")
    sint = nc.dram_tensor("sint", [128, L], FP16, kind="ExternalInput")
    jt = nc.dram_tensor("jt", [128, 128], FP16, kind="ExternalInput")
    triu = nc.dram_tensor("triu", [128, 128], FP16, kind="ExternalInput")
    ones1 = nc.dram_tensor("ones1", [1, 128], FP16, kind="ExternalInput")
    outt = nc.dram_tensor("outt", [L, E], FP16, kind="ExternalOutput")

    from contextlib import ExitStack

    with tile.TileContext(nc) as tc:
        with ExitStack() as ctx:
            ctx.enter_context(nc.allow_low_precision(
                reason="bf16 kernel; all contractions accumulate in f32 psum"))
            pool_specs = [
                ("consts", 1, None), ("res", 1, None),
                ("xt_p", 2, None), ("w1_p", 4, None),
                ("qt_p", 2, None), ("rq_p", 2, None), ("ckv_p", 2, None),
                ("pt_p", 6, None), ("tmp_p", 2, None),
                ("oz_p", 2, None), ("fin_p", 3, None),
                ("acc_p", 2, "PSUM"), ("rot_p", 4, "PSUM"),
            ]
            pools = {}
            for pname, pbufs, pspace in pool_specs:
                kw = {"name": pname, "bufs": pbufs}
                if pspace:
                    kw["space"] = pspace
                pools[pname] = ctx.enter_context(tc.tile_pool(**kw))
            (consts, res, xt_p, w1_p, qt_p, rq_p, ckv_p, pt_p, tmp_p,
             oz_p, fin_p, acc_p, rot_p) = (pools[s[0]] for s in pool_specs)

            def rot():
                return rot_p.tile([128, 512], F32, tag="ps", name="ps")

            # ---- constants / resident weights
            jt_t = consts.tile([128, 128], FP16, tag="jt", name="jt")
            nc.sync.dma_start(out=jt_t[:], in_=jt.ap())
            tri_t = consts.tile([128, 128], FP16, tag="tri", name="tri")
            nc.sync.dma_start(out=tri_t[:], in_=triu.ap())
            one_t = consts.tile([1, 128], FP16, tag="one", name="one")
            nc.sync.dma_start(out=one_t[:], in_=ones1.ap())
            wukt = res.tile([128, RK // 128, HPG * DH], FP16, tag="wukt", name="wukt")
            nc.sync.dma_start(out=wukt[:], in_=wuk.ap().rearrange("(t p) n -> p t n", p=128))
            wuvt = res.tile([128, RK // 128, HPG * DV], FP16, tag="wuvt", name="wuvt")
            nc.sync.dma_start(out=wuvt[:], in_=wuv.ap().rearrange("(t p) n -> p t n", p=128))
            wo_t = res.tile([128, 6, E], FP16, tag="wo", name="wo")
            nc.scalar.dma_start(out=wo_t[:], in_=wo.ap())
            cos_sb = res.tile([128, L], FP16, tag="cos", name="cos")
            nc.scalar.dma_start(out=cos_sb[:], in_=cost.ap())
            sin_sb = res.tile([128, L], FP16, tag="sin", name="sin")
            nc.scalar.dma_start(out=sin_sb[:], in_=sint.ap())

            ktc = res.tile([128, HPG, L], FP16, tag="ktc", name="ktc")   # K content, transposed
            rkd = res.tile([128, L], FP16, tag="rkd", name="rkd")        # roped k_rope, dup rows
            vd = res.tile([128, LT, VROW], FP16, tag="vd", name="vd")    # V resident (L-major + ones)
            vdv = vd[:].rearrange("p t (h x) -> p t h x", x=DV + 1)
            nc.gpsimd.memset(vdv[:, :, :, DV], 1.0)                      # ones columns

            # d-tiles of the fused QKV projection: (kind, idx)
            dtiles = ([("q", i) for i in range(HPG)]
                      + [("ckv", i) for i in range(RK // 128)]
                      + [("rq", i) for i in range(2)]
                      + [("rk", 0)])

            # deferred-work closures (prev chunk's norm tail + output proj),
            # emitted after the next chunk's QKV so the slow reciprocal and
            # the oz writes hide behind tensor-engine work
            pending = []

            for c in range(NCH):
                ccols = ds(c * CH, CH)

                # ================= QKV(c): [1344, CH] = W1^T @ x^T =======
                xtt = xt_p.tile([128, ET, CH], FP16, tag="xtt", name="xtt")
                nc.sync.dma_start(out=xtt[:], in_=xt.ap()[c])
                qtc = qt_p.tile([128, HPG, CH], FP16, tag="qtc", name="qtc")
                rq = rq_p.tile([128, 2, CH], FP16, tag="rq", name="rq")
                ckv = ckv_p.tile([128, RK // 128, CH], FP16, tag="ckv", name="ckv")

                for di, (kind, idx) in enumerate(dtiles):
                    w1s = w1_p.tile([128, ET, 128], FP16, tag="w1s", name="w1s")
                    nc.sync.dma_start(out=w1s[:], in_=w1.ap()[di])
                    dw = RD if kind == "rk" else 128
                    ps = rot()
                    for e in range(ET):
                        nc.tensor.matmul(ps[:dw, :CH], w1s[:, e, :dw], xtt[:, e, :],
                                         start=(e == 0), stop=(e == ET - 1))
                    if kind == "q":
                        nc.scalar.copy(out=qtc[:, idx, :], in_=ps[:, :CH])
                    elif kind == "ckv":
                        nc.vector.tensor_copy(ckv[:, idx, :], ps[:, :CH])
                    elif kind == "rq":
                        nc.vector.tensor_copy(rq[:, idx, :], ps[:, :CH])
                    else:  # pre-rope k_rope at partitions 0:64
                        nc.vector.tensor_copy(rkd[0:RD, ccols], ps[:RD, :CH])

                for fn in pending:
                    fn()
                pending = []

                # ================= RoPE(c) ===============================
                # roped = R * cos + (J @ R) * sin   (pairs along partitions)
                for i in range(2):  # q_rope, two head-pair tiles
                    swp = rot()
                    nc.tensor.matmul(swp[:, :CH], jt_t[:, :], rq[:, i, :],
                                     start=True, stop=True)
                    t1 = tmp_p.tile([128, CH], FP16, tag="ropet", name="ropet")
                    nc.vector.tensor_mul(t1[:], rq[:, i, :], cos_sb[:, ccols])
                    nc.vector.tensor_mul(rq[:, i, :], swp[:, :CH], sin_sb[:, ccols])
                    nc.vector.tensor_add(rq[:, i, :], rq[:, i, :], t1[:])
                swp = rot()
                nc.tensor.matmul(swp[:RD, :CH], jt_t[:RD, :RD], rkd[0:RD, ccols],
                                 start=True, stop=True)
                t1 = tmp_p.tile([128, CH], FP16, tag="ropet", name="ropet")
                nc.vector.tensor_mul(t1[:RD, :], rkd[0:RD, ccols], cos_sb[0:RD, ccols])
                nc.vector.tensor_mul(rkd[0:RD, ccols], swp[:RD, :CH], sin_sb[0:RD, ccols])
                nc.vector.tensor_add(rkd[0:RD, ccols], rkd[0:RD, ccols], t1[:RD, :])
                # duplicate roped k_rope to partitions 64:128 (for odd heads)
                nc.gpsimd.dma_start(out=rkd[RD:128, ccols], in_=rkd[0:RD, ccols])

                # ================= UP-K(c): K^T = Wuk^T @ c_kv^T =========
                for h in range(HPG):
                    ps = rot()
                    for kt in range(RK // 128):
                        nc.tensor.matmul(ps[:, :CH], wukt[:, kt, ds(128 * h, 128)],
                                         ckv[:, kt, :],
                                         start=(kt == 0), stop=(kt == RK // 128 - 1))
                    nc.scalar.copy(out=ktc[:, h, ccols], in_=ps[:, :CH])

                # ================= UP-V(c): V = c_kv @ Wuv (L-major) =====
                for lti in range(4):
                    lt = 4 * c + lti
                    for nb in range(2):
                        psv = rot()
                        for kt in range(RK // 128):
                            nc.tensor.matmul(psv[:, :384],
                                             ckv[:, kt, ds(128 * lti, 128)],
                                             wuvt[:, kt, ds(384 * nb, 384)],
                                             start=(kt == 0), stop=(kt == RK // 128 - 1))
                        for q in range(2):
                            hh = 2 * nb + q
                            nc.vector.tensor_copy(vd[:, lt, ds((DV + 1) * hh, DV)],
                                                  psv[:, ds(DV * q, DV)])

                # ================= ATT(c): head pairs, 1-tile pipelined ==
                ntk = 4 * c + 4
                oz = oz_p.tile([128, 6, CH], FP16, tag="oz", name="oz")

                def attn_half(hp, inject=None):
                    heads = (2 * hp, 2 * hp + 1)
                    A = [(acc_p.tile([128, 512], F32, tag="acc1", name="acc1"),
                          acc_p.tile([128, 512], F32, tag="acc2", name="acc2"))
                         for _ in range(2)]
                    geom = []
                    for t in range(ntk):
                        j = t - 4 * c
                        off = 128 * j if j >= 0 else 0
                        geom.append((off, CH - off, j >= 0))
                    sps_l = {}
                    pt_l = {}

                    def emit_qk(t):
                        off, n, _ = geom[t]
                        sps_l[t] = []
                        for q in range(2):
                            h = heads[q]
                            hb = RD * (h % 2)
                            sps = rot()
                            nc.tensor.matmul(sps[:, ds(off, n)],
                                             ktc[:, h, ds(128 * t, 128)],
                                             qtc[:, h, ds(off, n)],
                                             start=True, stop=False)
                            nc.tensor.matmul(sps[:, ds(off, n)],
                                             rkd[hb:hb + RD, ds(128 * t, 128)],
                                             rq[hb:hb + RD, h // 2, ds(off, n)],
                                             start=False, stop=True)
                            sps_l[t].append(sps)

                    def emit_exp(t):
                        off, n, diag = geom[t]
                        pt_l[t] = []
                        for q in range(2):
                            pt = pt_p.tile([128, CH], FP16, tag="pt", name="pt")
                            nc.scalar.activation(pt[:, ds(off, n)],
                                                 sps_l[t][q][:, ds(off, n)],
                                                 AF.Exp, scale=SCALE)
                            if diag:
                                nc.vector.tensor_mul(pt[:, ds(off, 128)],
                                                     pt[:, ds(off, 128)], tri_t[:])
                            pt_l[t].append(pt)

                    def emit_pv(t):
                        off, n, _ = geom[t]
                        for q in range(2):
                            h = heads[q]
                            ps1, ps2 = A[q]
                            pt = pt_l[t][q]
                            nc.tensor.matmul(ps1[:, ds(off, n)],
                                             vd[:, t, ds((DV + 1) * h, 128)],
                                             pt[:, ds(off, n)],
                                             start=(t == 0), stop=(t == ntk - 1),
                                             skip_group_check=True)
                            nc.tensor.matmul(ps2[:DV - DH + 1, ds(off, n)],
                                             vd[:, t, ds((DV + 1) * h + DH, DV - DH + 1)],
                                             pt[:, ds(off, n)],
                                             start=(t == 0), stop=(t == ntk - 1),
                                             skip_group_check=True)

                    # 1-tile software pipeline: QK(t+1) issues before PV(t)
                    emit_qk(0)
                    emit_exp(0)
                    for t in range(1, ntk):
                        emit_qk(t)
                        emit_pv(t - 1)
                        emit_exp(t)
                        if inject is not None and t == min(3, ntk - 1):
                            inject()
                            inject = None
                    emit_pv(ntk - 1)
                    if inject is not None:
                        inject()

                    # start the slow reciprocals now (DVE), defer the rest
                    rrs = []
                    for q in range(2):
                        rr16 = tmp_p.tile([1, CH], FP16, tag="rr16", name="rr16",
                                          bufs=4)
                        nc.vector.reciprocal(rr16[:], A[q][1][RD:RD + 1, :CH])
                        rrs.append(rr16)

                    def finish_norm():
                        for q in range(2):
                            h = heads[q]
                            ps1, ps2 = A[q]
                            rb = rot()
                            nc.tensor.matmul(rb[:, :CH], one_t[:, :], rrs[q][:, :],
                                             start=True, stop=True)
                            rbs = tmp_p.tile([128, CH], FP16, tag="rbs", name="rbs")
                            nc.scalar.copy(out=rbs[:], in_=rb[:, :CH])
                            nc.vector.tensor_mul(oz[:, h, :], ps1[:, :CH], rbs[:])
                            hb = RD * (h % 2)
                            nc.vector.tensor_mul(oz[hb:hb + RD, 4 + hp, :],
                                                 ps2[0:RD, :CH], rbs[0:RD, :])
                    return finish_norm

                fin0 = attn_half(0)
                fin1 = attn_half(1, inject=fin0)
                pending.append(fin1)

                # ===== FINAL(c): out = attn @ WO, deferred past QKV(c+1) =
                def make_final(c, oz):
                    def emit_final():
                        for eg in range(E // CH):
                            for ls in range(CH // 128):
                                fps = rot()
                                for kt in range(6):
                                    nc.tensor.matmul(fps[:, :CH],
                                                     oz[:, kt, ds(128 * ls, 128)],
                                                     wo_t[:, kt, ds(CH * eg, CH)],
                                                     start=(kt == 0), stop=(kt == 5))
                                fin = fin_p.tile([128, CH], FP16, tag="fin",
                                                 name="fin")
                                nc.scalar.copy(out=fin[:], in_=fps[:, :CH])
                                nc.gpsimd.dma_start(
                                    out=outt.ap()[ds(c * CH + 128 * ls, 128),
                                                  ds(CH * eg, CH)],
                                    in_=fin[:])
                    return emit_final

                pending.append(make_final(c, oz))

            for fn in pending:
                fn()
            pending = []

    _split_excess_waits(nc)
    return nc


def _prep_inputs(x, cos_table, sin_table, wq, wkv_down, w_up, w_out):
    f32 = np.float32
    wq3 = np.asarray(wq, f32).reshape(E, H, DV)
    wup3 = np.asarray(w_up, f32).reshape(RK, H, 2 * DH + RD)
    wo3 = np.asarray(w_out, f32).reshape(H, DV, E)
    wkv = np.asarray(wkv_down, f32)

    cosI = np.repeat(np.asarray(cos_table, f32)[:L], 2, axis=1).T  # [64, L]
    sinI = np.repeat(np.asarray(sin_table, f32)[:L], 2, axis=1).T
    cost = np.ascontiguousarray(np.concatenate([cosI, cosI], 0)).astype(NPFP16)
    sint = np.ascontiguousarray(np.concatenate([sinI, sinI], 0)).astype(NPFP16)
    J = np.zeros((128, 128), f32)
    for i in range(64):
        J[2 * i, 2 * i + 1] = -1.0
        J[2 * i + 1, 2 * i] = 1.0
    jt = np.ascontiguousarray(J.T).astype(NPFP16)
    triu = np.ascontiguousarray(np.triu(np.ones((128, 128), f32))).astype(NPFP16)

    in_maps = []
    for core in range(NCORE):
        b, g = core // HPG, core % HPG
        hs = slice(HPG * g, HPG * g + HPG)
        xT = np.asarray(x, f32)[b].T                       # [E, L]
        xt_pack = np.ascontiguousarray(
            xT.reshape(ET, 128, NCH, CH).transpose(2, 1, 0, 3)).astype(NPFP16)
        wq_c = wq3[:, hs, :DH].reshape(E, HPG * DH)
        wq_r = wq3[:, hs, DH:].reshape(E, HPG * RD)
        w1_flat = np.concatenate([wq_c, wkv[:, :RK], wq_r, wkv[:, RK:]], axis=1)
        # pack into 11 d-strips [128, ET, 128] (last strip: 64 cols, zero-pad)
        w1_pack = np.zeros((11, 128, ET, 128), f32)
        offs = [128 * i for i in range(10)] + [1280]
        wids = [128] * 10 + [64]
        for di, (o, w) in enumerate(zip(offs, wids)):
            w1_pack[di, :, :, :w] = (
                w1_flat[:, o:o + w].reshape(ET, 128, w).transpose(1, 0, 2))
        # w_out resident: 4 content strips + 2 rope-pair strips
        wo_pack = np.zeros((128, 6, E), f32)
        wog = wo3[hs]                                      # [4, 192, E]
        for kt in range(HPG):
            wo_pack[:, kt, :] = wog[kt, :DH, :]
        for hp in range(2):
            wo_pack[0:RD, 4 + hp, :] = wog[2 * hp, DH:, :]
            wo_pack[RD:128, 4 + hp, :] = wog[2 * hp + 1, DH:, :]
        in_maps.append({
            "xt": xt_pack,
            "w1": w1_pack.astype(NPFP16),
            "wuk": np.ascontiguousarray(
                wup3[:, hs, :DH].reshape(RK, HPG * DH)).astype(NPFP16),
            "wuv": np.ascontiguousarray(
                wup3[:, hs, DH:].reshape(RK, HPG * DV)).astype(NPFP16),
            "wo": wo_pack.astype(NPFP16),
            "cost": cost,
            "sint": sint,
            "jt": jt,
            "triu": triu,
            "ones1": np.ones((1, 128), NPFP16),
        })
    return in_maps


def kernel(x, cos_table, sin_table, wq, wkv_down, w_up, w_out, _want_perf=False):
    if "nc" not in _CACHE:
        _CACHE["nc"] = _build()
    nc = _CACHE["nc"]
    in_maps = _prep_inputs(x, cos_table, sin_table, wq, wkv_down, w_up, w_out)
    res = run_bass_kernel_spmd(nc, in_maps, core_ids=list(range(NCORE)),
                               trace=bool(_want_perf),
                               tmpdir=os.environ.get("BASS_TMPDIR") or None)
    out = np.zeros((B, L, E), np.float32)
    for core in range(NCORE):
        b = core // HPG
        out[b] += res.results[core]["outt"].astype(np.float32)
    if _want_perf:
        return out, res
    return out


# revision 19
# speedup vs baseline: 2.3941x; 1.0255x over previous
"""MLA prefill kernel (fp16) for Trainium2, 8 NeuronCores.

Sharding: data-parallel over batch (2) x tensor-parallel over heads
(16 heads -> 4 per core).  Core c handles batch c//4, head group c%4.
Each core computes its full attention block plus a partial output
projection; the host sums the 4 per-group partials per batch.

All matmul operands are bf16 (1 cycle/row on the PE; fp32/f32r ran in
fp32_mode=HIGH at ~2.5 cycles/row), accumulation stays f32 in PSUM.
Everything is computed transposed ([feature, L]) so matmul lhsT/rhs
operands are produced directly, except V (L-major for the PV matmul),
which stays resident in SBUF.  Scores are computed transposed
(S^T = K Q^T, [Lk, Lq]) so softmax's sum runs through the PV matmul
via an appended ones-column; exp needs no max-subtraction (scores are
O(10)).  RoPE pair mixing runs along partitions via a +-1 pair-swap
matmul (J) plus two elementwise multiplies and an add.

The attention inner loop is software-pipelined one key-tile ahead
(QK(t+1) issues before PV(t)) so the tensor engine does not stall on
the exp activation.  PSUM: 4 banks hold the two heads' PV accumulators,
4 banks rotate for scores/projections.  The softmax denominator row is
reciprocal'd on the DVE and broadcast across partitions with a K=1
ones matmul.  w_out is SBUF-resident with the per-head rope halves
packed in pairs so the output projection runs 6 full-K=128 matmuls.
"""

import math
import os
import sys

sys.path.insert(0, "/opt/trn_rl_repo")

import numpy as np
import ml_dtypes

import concourse.bass as bass
import concourse.mybir as mybir
import concourse.tile as tile
from concourse.bass import ds
from concourse.bass_utils import run_bass_kernel_spmd

H, DH, RK, RD = 16, 128, 512, 64
B, L, E = 2, 2048, 2048
HPG = 4                      # heads per core
NCORE = 8
DV = DH + RD                 # 192
SCALE = 1.0 / math.sqrt(DV)
CH = 512                     # Lq chunk
NCH = L // CH                # 4
LT = L // 128                # 16 key tiles
ET = E // 128                # 16
VROW = HPG * (DV + 1)        # 772: per-head 192 v dims + ones col

F32 = mybir.dt.float32
FP16 = mybir.dt.float16
AF = mybir.ActivationFunctionType
NPFP16 = np.float16

_CACHE = {}


def _split_excess_waits(nc, limit=1):
    """walrus on this toolchain accepts at most one sem-wait per
    instruction; hoist extras onto same-engine no-ops just before."""
    f = nc.m.functions[0]
    for bb in f.blocks:
        new_list = []
        changed = False
        for inst in bb.instructions:
            si = inst.sync_info
            if si is not None and si.on_wait is not None and len(si.on_wait) > limit:
                waits = list(si.on_wait)
                changed = True
                n = 0
                while len(waits) > limit:
                    chunk, waits = waits[:limit], waits[limit:]
                    new_list.append(mybir.InstNoOp(
                        name=f"{inst.name}-ws{n}",
                        sync_info=mybir.SyncInfo(on_wait=chunk, on_update=[]),
                        bass_nofuse=True,
                        engine=inst.engine,
                    ))
                    n += 1
                inst.sync_info = mybir.SyncInfo(on_wait=waits, on_update=si.on_update)
            new_list.append(inst)
        if changed:
            bb.instructions[:] = new_list
    return nc


def _build():
    nc = bass.Bass(target_bir_lowering=False, trn_type="TRN2")

    xt = nc.dram_tensor("xt", [NCH, 128, ET, CH], FP16, kind="ExternalInput")
    w1 = nc.dram_tensor("w1", [11, 128, ET, 128], FP16, kind="ExternalInput")
    wuk = nc.dram_tensor("wuk", [RK, HPG * DH], FP16, kind="ExternalInput")
    wuv = nc.dram_tensor("wuv", [RK, HPG * DV], FP16, kind="ExternalInput")
    wo = nc.dram_tensor("wo", [128, 6, E], FP16, kind="ExternalInput")
    cost = nc.dram_tensor("cost", [128, L], FP16, kind="ExternalInput")
    sint = nc.dram_tensor("sint", [128, L], FP16, kind="ExternalInput")
    jt = nc.dram_tensor("jt", [128, 128], FP16, kind="ExternalInput")
    triu = nc.dram_tensor("triu", [128, 128], FP16, kind="ExternalInput")
    ones1 = nc.dram_tensor("ones1", [1, 128], FP16, kind="ExternalInput")
    outt = nc.dram_tensor("outt", [L, E], FP16, kind="ExternalOutput")

    from contextlib import ExitStack

    with tile.TileContext(nc) as tc:
        with ExitStack() as ctx:
            ctx.enter_context(nc.allow_low_precision(
                reason="bf16 kernel; all contractions accumulate in f32 psum"))
            pool_specs = [
                ("consts", 1, None), ("res", 1, None),
                ("xt_p", 2, None), ("w1_p", 4, None),
                ("qt_p", 2, None), ("rq_p", 2, None), ("ckv_p", 2, None),
                ("pt_p", 6, None), ("tmp_p", 2, None),
                ("oz_p", 2, None), ("fin_p", 3, None),
                ("acc_p", 2, "PSUM"), ("rot_p", 4, "PSUM"),
            ]
            pools = {}
            for pname, pbufs, pspace in pool_specs:
                kw = {"name": pname, "bufs": pbufs}
                if pspace:
                    kw["space"] = pspace
                pools[pname] = ctx.enter_context(tc.tile_pool(**kw))
            (consts, res, xt_p, w1_p, qt_p, rq_p, ckv_p, pt_p, tmp_p,
             oz_p, fin_p, acc_p, rot_p) = (pools[s[0]] for s in pool_specs)

            def rot():
                return rot_p.tile([128, 512], F32, tag="ps", name="ps")

            # ---- constants / resident weights.  Only the small consts and
            # cos/sin go ahead of chunk 0's xt/w1 input DMAs; the bulky
            # resident weights (wuk/wuv/wo) are dispatched after chunk 0's
            # QKV emission so the first matmul isn't stuck behind them.
            jt_t = consts.tile([128, 128], FP16, tag="jt", name="jt")
            nc.sync.dma_start(out=jt_t[:], in_=jt.ap())
            tri_t = consts.tile([128, 128], FP16, tag="tri", name="tri")
            nc.sync.dma_start(out=tri_t[:], in_=triu.ap())
            one_t = consts.tile([1, 128], FP16, tag="one", name="one")
            nc.sync.dma_start(out=one_t[:], in_=ones1.ap())
            cos_sb = res.tile([128, L], FP16, tag="cos", name="cos")
            nc.scalar.dma_start(out=cos_sb[:], in_=cost.ap())
            sin_sb = res.tile([128, L], FP16, tag="sin", name="sin")
            nc.scalar.dma_start(out=sin_sb[:], in_=sint.ap())
            wukt = res.tile([128, RK // 128, HPG * DH], FP16, tag="wukt", name="wukt")
            wuvt = res.tile([128, RK // 128, HPG * DV], FP16, tag="wuvt", name="wuvt")
            wo_t = res.tile([128, 6, E], FP16, tag="wo", name="wo")

            def load_residents():
                nc.sync.dma_start(
                    out=wukt[:], in_=wuk.ap().rearrange("(t p) n -> p t n", p=128))
                nc.sync.dma_start(
                    out=wuvt[:], in_=wuv.ap().rearrange("(t p) n -> p t n", p=128))
                nc.scalar.dma_start(out=wo_t[:], in_=wo.ap())

            ktc = res.tile([128, HPG, L], FP16, tag="ktc", name="ktc")   # K content, transposed
            rkd = res.tile([128, L], FP16, tag="rkd", name="rkd")        # roped k_rope, dup rows
            vd = res.tile([128, LT, VROW], FP16, tag="vd", name="vd")    # V resident (L-major + ones)
            vdv = vd[:].rearrange("p t (h x) -> p t h x", x=DV + 1)
            nc.gpsimd.memset(vdv[:, :, :, DV], 1.0)                      # ones columns

            # d-tiles of the fused QKV projection: (kind, idx)
            dtiles = ([("q", i) for i in range(HPG)]
                      + [("ckv", i) for i in range(RK // 128)]
                      + [("rq", i) for i in range(2)]
                      + [("rk", 0)])

            # deferred-work closures (prev chunk's norm tail + output proj),
            # emitted after the next chunk's QKV so the slow reciprocal and
            # the oz writes hide behind tensor-engine work
            pending = []

            for c in range(NCH):
                ccols = ds(c * CH, CH)

                # ================= QKV(c): [1344, CH] = W1^T @ x^T =======
                xtt = xt_p.tile([128, ET, CH], FP16, tag="xtt", name="xtt")
                nc.sync.dma_start(out=xtt[:], in_=xt.ap()[c])
                qtc = qt_p.tile([128, HPG, CH], FP16, tag="qtc", name="qtc")
                rq = rq_p.tile([128, 2, CH], FP16, tag="rq", name="rq")
                ckv = ckv_p.tile([128, RK // 128, CH], FP16, tag="ckv", name="ckv")

                for di, (kind, idx) in enumerate(dtiles):
                    w1s = w1_p.tile([128, ET, 128], FP16, tag="w1s", name="w1s")
                    nc.sync.dma_start(out=w1s[:], in_=w1.ap()[di])
                    dw = RD if kind == "rk" else 128
                    ps = rot()
                    for e in range(ET):
                        nc.tensor.matmul(ps[:dw, :CH], w1s[:, e, :dw], xtt[:, e, :],
                                         start=(e == 0), stop=(e == ET - 1))
                    if kind == "q":
                        nc.scalar.copy(out=qtc[:, idx, :], in_=ps[:, :CH])
                    elif kind == "ckv":
                        nc.vector.tensor_copy(ckv[:, idx, :], ps[:, :CH])
                    elif kind == "rq":
                        nc.vector.tensor_copy(rq[:, idx, :], ps[:, :CH])
                    else:  # pre-rope k_rope at partitions 0:64
                        nc.vector.tensor_copy(rkd[0:RD, ccols], ps[:RD, :CH])

                if c == 0:
                    load_residents()
                for fn in pending:
                    fn()
                pending = []

                # ================= RoPE(c) ===============================
                # roped = R * cos + (J @ R) * sin   (pairs along partitions)
                for i in range(2):  # q_rope, two head-pair tiles
                    swp = rot()
                    nc.tensor.matmul(swp[:, :CH], jt_t[:, :], rq[:, i, :],
                                     start=True, stop=True)
                    t1 = tmp_p.tile([128, CH], FP16, tag="ropet", name="ropet")
                    nc.vector.tensor_mul(t1[:], rq[:, i, :], cos_sb[:, ccols])
                    nc.vector.tensor_mul(rq[:, i, :], swp[:, :CH], sin_sb[:, ccols])
                    nc.vector.tensor_add(rq[:, i, :], rq[:, i, :], t1[:])
                swp = rot()
                nc.tensor.matmul(swp[:RD, :CH], jt_t[:RD, :RD], rkd[0:RD, ccols],
                                 start=True, stop=True)
                t1 = tmp_p.tile([128, CH], FP16, tag="ropet", name="ropet")
                nc.vector.tensor_mul(t1[:RD, :], rkd[0:RD, ccols], cos_sb[0:RD, ccols])
                nc.vector.tensor_mul(rkd[0:RD, ccols], swp[:RD, :CH], sin_sb[0:RD, ccols])
                nc.vector.tensor_add(rkd[0:RD, ccols], rkd[0:RD, ccols], t1[:RD, :])
                # duplicate roped k_rope to partitions 64:128 (for odd heads)
                nc.sync.dma_start(out=rkd[RD:128, ccols], in_=rkd[0:RD, ccols])

                # ================= UP-K(c): K^T = Wuk^T @ c_kv^T =========
                for h in range(HPG):
                    ps = rot()
                    for kt in range(RK // 128):
                        nc.tensor.matmul(ps[:, :CH], wukt[:, kt, ds(128 * h, 128)],
                                         ckv[:, kt, :],
                                         start=(kt == 0), stop=(kt == RK // 128 - 1))
                    nc.scalar.copy(out=ktc[:, h, ccols], in_=ps[:, :CH])

                # ================= UP-V(c): V = c_kv @ Wuv (L-major) =====
                for lti in range(4):
                    lt = 4 * c + lti
                    for nb in range(2):
                        psv = rot()
                        for kt in range(RK // 128):
                            nc.tensor.matmul(psv[:, :384],
                                             ckv[:, kt, ds(128 * lti, 128)],
                                             wuvt[:, kt, ds(384 * nb, 384)],
                                             start=(kt == 0), stop=(kt == RK // 128 - 1))
                        for q in range(2):
                            hh = 2 * nb + q
                            nc.vector.tensor_copy(vd[:, lt, ds((DV + 1) * hh, DV)],
                                                  psv[:, ds(DV * q, DV)])

                # ================= ATT(c): head pairs, 1-tile pipelined ==
                ntk = 4 * c + 4
                oz = oz_p.tile([128, 6, CH], FP16, tag="oz", name="oz")

                def attn_half(hp, inject=None):
                    heads = (2 * hp, 2 * hp + 1)
                    A = [(acc_p.tile([128, 512], F32, tag="acc1", name="acc1"),
                          acc_p.tile([128, 512], F32, tag="acc2", name="acc2"))
                         for _ in range(2)]
                    geom = []
                    for t in range(ntk):
                        j = t - 4 * c
                        off = 128 * j if j >= 0 else 0
                        geom.append((off, CH - off, j >= 0))
                    sps_l = {}
                    pt_l = {}

                    def emit_qk(t):
                        off, n, _ = geom[t]
                        sps_l[t] = []
                        for q in range(2):
                            h = heads[q]
                            hb = RD * (h % 2)
                            sps = rot()
                            nc.tensor.matmul(sps[:, ds(off, n)],
                                             ktc[:, h, ds(128 * t, 128)],
                                             qtc[:, h, ds(off, n)],
                                             start=True, stop=False)
                            nc.tensor.matmul(sps[:, ds(off, n)],
                                             rkd[hb:hb + RD, ds(128 * t, 128)],
                                             rq[hb:hb + RD, h // 2, ds(off, n)],
                                             start=False, stop=True)
                            sps_l[t].append(sps)

                    def emit_exp(t):
                        off, n, diag = geom[t]
                        pt_l[t] = []
                        for q in range(2):
                            pt = pt_p.tile([128, CH], FP16, tag="pt", name="pt")
                            nc.scalar.activation(pt[:, ds(off, n)],
                                                 sps_l[t][q][:, ds(off, n)],
                                                 AF.Exp, scale=SCALE)
                            if diag:
                                # on Pool: keeps the mask off the DVE, whose
                                # queue carries the slow reciprocals
                                nc.gpsimd.tensor_mul(pt[:, ds(off, 128)],
                                                     pt[:, ds(off, 128)], tri_t[:])
                            pt_l[t].append(pt)

                    def emit_pv(t):
                        off, n, _ = geom[t]
                        for q in range(2):
                            h = heads[q]
                            ps1, ps2 = A[q]
                            pt = pt_l[t][q]
                            nc.tensor.matmul(ps1[:, ds(off, n)],
                                             vd[:, t, ds((DV + 1) * h, 128)],
                                             pt[:, ds(off, n)],
                                             start=(t == 0), stop=(t == ntk - 1),
                                             skip_group_check=True)
                            nc.tensor.matmul(ps2[:DV - DH + 1, ds(off, n)],
                                             vd[:, t, ds((DV + 1) * h + DH, DV - DH + 1)],
                                             pt[:, ds(off, n)],
                                             start=(t == 0), stop=(t == ntk - 1),
                                             skip_group_check=True)

                    # 2-tile software pipeline: PV(t) trails QK(t+2) so the
                    # exp (and diag mask) latency is fully covered by PE work
                    emit_qk(0)
                    emit_qk(1)
                    emit_exp(0)
                    for t in range(2, ntk):
                        emit_qk(t)
                        emit_pv(t - 2)
                        emit_exp(t - 1)
                        if inject is not None and t == min(3, ntk - 1):
                            inject()
                            inject = None
                    emit_pv(ntk - 2)
                    emit_exp(ntk - 1)
                    emit_pv(ntk - 1)
                    if inject is not None:
                        inject()

                    # start the slow reciprocals now (DVE), defer the rest
                    rrs = []
                    for q in range(2):
                        rr16 = tmp_p.tile([1, CH], FP16, tag="rr16", name="rr16",
                                          bufs=4)
                        nc.vector.reciprocal(rr16[:], A[q][1][RD:RD + 1, :CH])
                        rrs.append(rr16)

                    def finish_norm():
                        for q in range(2):
                            h = heads[q]
                            ps1, ps2 = A[q]
                            rb = rot()
                            nc.tensor.matmul(rb[:, :CH], one_t[:, :], rrs[q][:, :],
                                             start=True, stop=True)
                            rbs = tmp_p.tile([128, CH], FP16, tag="rbs", name="rbs")
                            nc.scalar.copy(out=rbs[:], in_=rb[:, :CH])
                            nc.vector.tensor_mul(oz[:, h, :], ps1[:, :CH], rbs[:])
                            hb = RD * (h % 2)
                            nc.vector.tensor_mul(oz[hb:hb + RD, 4 + hp, :],
                                                 ps2[0:RD, :CH], rbs[0:RD, :])
                    return finish_norm

                fin0 = attn_half(0)
                fin1 = attn_half(1, inject=fin0)
                pending.append(fin1)

                # ===== FINAL(c): out = attn @ WO, deferred past QKV(c+1) =
                def make_final(c, oz):
                    def emit_final():
                        for eg in range(E // CH):
                            for ls in range(CH // 128):
                                fps = rot()
                                for kt in range(6):
                                    nc.tensor.matmul(fps[:, :CH],
                                                     oz[:, kt, ds(128 * ls, 128)],
                                                     wo_t[:, kt, ds(CH * eg, CH)],
                                                     start=(kt == 0), stop=(kt == 5))
                                fin = fin_p.tile([128, CH], FP16, tag="fin",
                                                 name="fin")
                                nc.scalar.copy(out=fin[:], in_=fps[:, :CH])
                                nc.gpsimd.dma_start(
                                    out=outt.ap()[ds(c * CH + 128 * ls, 128),
                                                  ds(CH * eg, CH)],
                                    in_=fin[:])
                    return emit_final

                pending.append(make_final(c, oz))

            for fn in pending:
                fn()
            pending = []

    _split_excess_waits(nc)
    return nc


def _prep_inputs(x, cos_table, sin_table, wq, wkv_down, w_up, w_out):
    f32 = np.float32
    wq3 = np.asarray(wq, f32).reshape(E, H, DV)
    wup3 = np.asarray(w_up, f32).reshape(RK, H, 2 * DH + RD)
    wo3 = np.asarray(w_out, f32).reshape(H, DV, E)
    wkv = np.asarray(wkv_down, f32)

    cosI = np.repeat(np.asarray(cos_table, f32)[:L], 2, axis=1).T  # [64, L]
    sinI = np.repeat(np.asarray(sin_table, f32)[:L], 2, axis=1).T
    cost = np.ascontiguousarray(np.concatenate([cosI, cosI], 0)).astype(NPFP16)
    sint = np.ascontiguousarray(np.concatenate([sinI, sinI], 0)).astype(NPFP16)
    J = np.zeros((128, 128), f32)
    for i in range(64):
        J[2 * i, 2 * i + 1] = -1.0
        J[2 * i + 1, 2 * i] = 1.0
    jt = np.ascontiguousarray(J.T).astype(NPFP16)
    triu = np.ascontiguousarray(np.triu(np.ones((128, 128), f32))).astype(NPFP16)

    in_maps = []
    for core in range(NCORE):
        b, g = core // HPG, core % HPG
        hs = slice(HPG * g, HPG * g + HPG)
        xT = np.asarray(x, f32)[b].T                       # [E, L]
        xt_pack = np.ascontiguousarray(
            xT.reshape(ET, 128, NCH, CH).transpose(2, 1, 0, 3)).astype(NPFP16)
        wq_c = wq3[:, hs, :DH].reshape(E, HPG * DH)
        wq_r = wq3[:, hs, DH:].reshape(E, HPG * RD)
        w1_flat = np.concatenate([wq_c, wkv[:, :RK], wq_r, wkv[:, RK:]], axis=1)
        # pack into 11 d-strips [128, ET, 128] (last strip: 64 cols, zero-pad)
        w1_pack = np.zeros((11, 128, ET, 128), f32)
        offs = [128 * i for i in range(10)] + [1280]
        wids = [128] * 10 + [64]
        for di, (o, w) in enumerate(zip(offs, wids)):
            w1_pack[di, :, :, :w] = (
                w1_flat[:, o:o + w].reshape(ET, 128, w).transpose(1, 0, 2))
        # w_out resident: 4 content strips + 2 rope-pair strips
        wo_pack = np.zeros((128, 6, E), f32)
        wog = wo3[hs]                                      # [4, 192, E]
        for kt in range(HPG):
            wo_pack[:, kt, :] = wog[kt, :DH, :]
        for hp in range(2):
            wo_pack[0:RD, 4 + hp, :] = wog[2 * hp, DH:, :]
            wo_pack[RD:128, 4 + hp, :] = wog[2 * hp + 1, DH:, :]
        in_maps.append({
            "xt": xt_pack,
            "w1": w1_pack.astype(NPFP16),
            "wuk": np.ascontiguousarray(
                wup3[:, hs, :DH].reshape(RK, HPG * DH)).astype(NPFP16),
            "wuv": np.ascontiguousarray(
                wup3[:, hs, DH:].reshape(RK, HPG * DV)).astype(NPFP16),
            "wo": wo_pack.astype(NPFP16),
            "cost": cost,
            "sint": sint,
            "jt": jt,
            "triu": triu,
            "ones1": np.ones((1, 128), NPFP16),
        })
    return in_maps


def kernel(x, cos_table, sin_table, wq, wkv_down, w_up, w_out, _want_perf=False):
    if "nc" not in _CACHE:
        _CACHE["nc"] = _build()
    nc = _CACHE["nc"]
    in_maps = _prep_inputs(x, cos_table, sin_table, wq, wkv_down, w_up, w_out)
    res = run_bass_kernel_spmd(nc, in_maps, core_ids=list(range(NCORE)),
                               trace=bool(_want_perf),
                               tmpdir=os.environ.get("BASS_TMPDIR") or None)
    out = np.zeros((B, L, E), np.float32)
    for core in range(NCORE):
        b = core // HPG
        out[b] += res.results[core]["outt"].astype(np.float32)
    if _want_perf:
        return out, res
    return out
